# revision 20
# baseline (speedup 1.0000x reference)
"""Trainium2 Bass kernel for nn_Block_69578470195514 (dense transformer block).

Contract: kernel(**inputs) takes the FULL unsharded inputs (B=8,T=1024,D=1024,
H=16) as numpy arrays and returns the FULL [8,1024,1024] float32 output.

Sharding: pure data-parallel over batch - core b processes batch element b.
Weights are replicated. No collectives.

The kernel is HBM-bandwidth-bound (device aggregate ~290GB/s over 8 cores),
so the design minimizes per-iteration DRAM traffic (~22MB/core):
  x upload bf16 (2MB), out store bf16 (2MB),
  Wq/Wk/Wv/Wp stored fp8 e4m3 scaled x16 (4MB total),
  W1/W2 bf16 (16MB) streamed chunk-interleaved into a k-outer FF2 so the
  stream is consumed as it arrives (no big-resident W2, tiny compute tail).

Scale folding for the x16 fp8 weights: q_s=16q, k_s=16k (bias x16), so
scores_s=256*scores -> exp scale = D^-0.5/256. v_s=16v (bias x16); the
per-row softmax normalizer multiply also folds 1/256 so att_s=att/16;
proj with Wp_s=16*Wp then gives the exact attention projection in PSUM.

On-chip dtypes: xnT/qT/kT/attT fp8 (saves SBUF for the W1/W2 staging that
keeps the DMA queue busy during attention), v/x2/xn2T/h bf16, PSUM fp32.
"""

import numpy as np
import ml_dtypes

BF16 = ml_dtypes.bfloat16
E4M3 = ml_dtypes.float8_e4m3

P = 128
B, T, D, H = 8, 1024, 1024, 16
DH = D // H
F = 4 * D
NT = T // P   # 8 token tiles
ND = D // P   # 8 feature tiles
NF = F // P   # 32 ff tiles
GK = 16       # FF2 k-group size (chunks held resident per group)
EPS = 1e-3
SCALE = float(D) ** -0.5
WS = 16.0     # fp8 weight pre-scale
MASKVAL = -1.0e6

_cache = {}


def _split_multiwait_insts(nc, mybir):
    """This walrus build allows only 1 sync-wait per instruction. Tile can
    attach several. Hoist all but the last wait of any instruction into
    preceding single-wait InstEventSemaphore carriers on the same engine."""
    for bb in nc.main_func.blocks:
        insts = list(bb.instructions)
        out = []
        changed = False
        for inst in insts:
            si = inst.sync_info
            if si is not None and si.on_wait and len(si.on_wait) > 1:
                waits = list(si.on_wait)
                for k, w in enumerate(waits[:-1]):
                    d = mybir.InstEventSemaphore(
                        name=f"{inst.name}_wsplit{k}", ins=[], outs=[]
                    )
                    d.engine = inst.engine
                    d.sync_info = mybir.SyncInfo(on_wait=[w], on_update=[])
                    out.append(d)
                inst.sync_info = mybir.SyncInfo(
                    on_wait=[waits[-1]], on_update=list(si.on_update)
                )
                changed = True
            out.append(inst)
        if changed:
            try:
                bb.instructions[:] = out
            except Exception:
                bb.instructions.clear()
                for i in out:
                    bb.add_instruction(i)


def _av_chunks(r0):
    """Column chunks for the AV matmuls of s-tile starting at r0, split on
    PSUM bank boundaries (512 fp32)."""
    chunks = []
    for b0 in range(0, T, 512):
        lo = max(r0, b0)
        hi = b0 + 512
        if lo < hi:
            chunks.append((lo, hi))
    return chunks


def _build(reps=1, has_bp=False, has_b2=False, has_qkvb=True, has_b1=True):
    from contextlib import ExitStack

    import concourse.bass as bass
    import concourse.tile as tile
    import concourse.mybir as mybir

    f32 = mybir.dt.float32
    bf16 = mybir.dt.bfloat16
    fp8 = mybir.dt.float8e4
    AF = mybir.ActivationFunctionType
    ALU = mybir.AluOpType

    nc = bass.Bass()

    x_d = nc.dram_tensor("x", [T, D], bf16, kind="ExternalInput")
    wq_d = nc.dram_tensor("wq", [D, D], fp8, kind="ExternalInput")
    wk_d = nc.dram_tensor("wk", [D, D], fp8, kind="ExternalInput")
    wv_d = nc.dram_tensor("wv", [D, D], fp8, kind="ExternalInput")
    wp_d = nc.dram_tensor("wp", [D, D], fp8, kind="ExternalInput")
    # w1 is host-chunked: w1[kc*128+p, k*128+j] = (g2*W1)[k*128+p, kc*128+j]
    # so each [P, ND*P] chunk DMA has contiguous 2KB per-partition runs.
    w1_d = nc.dram_tensor("w1", [F, D], bf16, kind="ExternalInput")
    w2_d = nc.dram_tensor("w2", [F, D], bf16, kind="ExternalInput")
    b1_d = qb_d = kb_d = vb_d = None
    if has_b1:
        b1_d = nc.dram_tensor("b1t", [P, NF], f32, kind="ExternalInput")
    if has_qkvb:
        qb_d = nc.dram_tensor("qbt", [P, ND], f32, kind="ExternalInput")
        kb_d = nc.dram_tensor("kbt", [P, ND], f32, kind="ExternalInput")
        vb_d = nc.dram_tensor("vbr", [1, D], f32, kind="ExternalInput")
    mask_d = nc.dram_tensor("mask", [P, P], f32, kind="ExternalInput")
    id_d = nc.dram_tensor("ident", [P, P], bf16, kind="ExternalInput")
    if has_bp:
        bp_d = nc.dram_tensor("bpr", [1, D], f32, kind="ExternalInput")
    if has_b2:
        b2_d = nc.dram_tensor("b2r", [1, D], f32, kind="ExternalInput")
    out_d = nc.dram_tensor("out", [T, D], bf16, kind="ExternalOutput")

    def bcast(ap_1d):
        # [1, N] dram row -> broadcast across partitions
        return bass.AP(
            tensor=ap_1d.tensor,
            offset=ap_1d.offset,
            ap=[[0, P]] + list(ap_1d.ap)[1:],
        )

    with tile.TileContext(nc, pool_alloc_mode="queue") as tc, ExitStack() as top:
        const = top.enter_context(tc.tile_pool(name="const", bufs=1))
        mask_sb = const.tile([P, P], f32)
        id_sb = const.tile([P, P], bf16)
        b1_sb = qb_sb = kb_sb = vb_sb = None
        if has_b1:
            b1_sb = const.tile([P, NF], f32)
        if has_qkvb:
            qb_sb = const.tile([P, ND], f32)
            kb_sb = const.tile([P, ND], f32)
            vb_sb = const.tile([P, D], f32)
        eps_sb = const.tile([P, 1], f32)
        nc.vector.memset(eps_sb, EPS)
        # exp bias -ln(WS^2): emits exp(z)/256 so wexp/sums stay in range
        # while qT/kT carry the x16 fp8 scale.
        lnb_sb = const.tile([P, 1], f32)
        nc.vector.memset(lnb_sb, -float(np.log(WS * WS)))
        bp_sb = b2_sb = None
        if has_bp:
            bp_sb = const.tile([P, D], f32)
        if has_b2:
            b2_sb = const.tile([P, D], f32)

        def const_dmas():
            nc.sync.dma_start(out=id_sb, in_=id_d[:, :])
            nc.sync.dma_start(out=mask_sb, in_=mask_d[:, :])
            if b1_sb is not None:
                nc.sync.dma_start(out=b1_sb, in_=b1_d[:, :])
            if qb_sb is not None:
                nc.sync.dma_start(out=qb_sb, in_=qb_d[:, :])
                nc.sync.dma_start(out=kb_sb, in_=kb_d[:, :])
                nc.sync.dma_start(out=vb_sb, in_=bcast(vb_d[:, :]))
            if bp_sb is not None:
                nc.sync.dma_start(out=bp_sb, in_=bcast(bp_d[:, :]))
            if b2_sb is not None:
                nc.sync.dma_start(out=b2_sb, in_=bcast(b2_d[:, :]))

        emit_args = (
            nc, tc, tile, bass, mybir, f32, bf16, fp8, AF, ALU,
            x_d, wq_d, wk_d, wv_d, wp_d, w1_d, w2_d, out_d,
            mask_sb, id_sb, b1_sb, qb_sb, kb_sb, vb_sb, eps_sb, lnb_sb,
            bp_sb, b2_sb, const_dmas,
        )
        if reps == 1:
            _emit(*emit_args)
        else:
            with tc.For_i(0, reps, 1):
                _emit(*emit_args)

    _split_multiwait_insts(nc, mybir)
    return nc


def _emit(
    nc, tc, tile, bass, mybir, f32, bf16, fp8, AF, ALU,
    x_d, wq_d, wk_d, wv_d, wp_d, w1_d, w2_d, out_d,
    mask_sb, id_sb, b1_sb, qb_sb, kb_sb, vb_sb, eps_sb, lnb_sb, bp_sb, b2_sb,
    const_dmas,
):
    from contextlib import ExitStack

    def ln_tile(stats, xin, xcout, tags):
        st = stats.tile([P, 2, 6], f32, tag=tags + "st")
        nc.vector.bn_stats(out=st[:, 0, :], in_=xin[:, 0:512])
        nc.vector.bn_stats(out=st[:, 1, :], in_=xin[:, 512:1024])
        mv = stats.tile([P, 2], f32, tag=tags + "mv")
        nc.vector.bn_aggr(out=mv, in_=st)
        sd = stats.tile([P, 1], f32, tag=tags + "sd")
        nc.scalar.activation(sd, mv[:, 1:2], AF.Sqrt, bias=eps_sb)
        rs = stats.tile([P, 1], f32, tag=tags + "rs")
        nc.vector.reciprocal(out=rs, in_=sd)
        nmu = stats.tile([P, 1], f32, tag=tags + "nmu")
        nc.vector.tensor_scalar(
            out=nmu, in0=mv[:, 0:1], scalar1=rs, scalar2=-1.0,
            op0=ALU.mult, op1=ALU.mult,
        )
        # (x - mu) * rsig on ACT: Identity(x*rs + (-mu*rs))
        nc.scalar.activation(xcout, xin, AF.Identity, bias=nmu, scale=rs)

    with ExitStack() as ctx:
        # Long-lived arrays on the RIGHT allocation stack.
        pR1 = ctx.enter_context(tc.tile_pool(name="pR1", bufs=1, side="right"))
        x2 = pR1.tile([P, NT, D], bf16)       # residual stream 2 [t, d]
        pR2 = ctx.enter_context(tc.tile_pool(name="pR2", bufs=1, side="right"))
        xn2T = pR2.tile([P, ND, T], bf16)     # ln2(x2)^T [d, t]
        pX = ctx.enter_context(tc.tile_pool(name="pX", bufs=1, side="right"))
        x_sb = pX.tile([P, NT, D], bf16)      # resident input x [t, d]

        # FF weight staging pools: sized so the DMA queue never stalls while
        # attention computes; w2 chunks of a k-group stay alive through FF2.
        w1p = ctx.enter_context(tc.tile_pool(name="w1p", bufs=8))
        w2p = ctx.enter_context(tc.tile_pool(name="w2p", bufs=GK + 2))

        # ======== phases A..C scope ========
        with ExitStack() as pab:
            pA = pab.enter_context(tc.tile_pool(name="pA", bufs=1))
            xnT = pA.tile([P, ND, T], fp8)    # ln1(x)^T  [d, t]
            qT = pA.tile([P, ND, T], fp8)     # [e, t] (x16)
            kT = pA.tile([P, ND, T], fp8)     # [e, s] (x16)
            v = pA.tile([P, NT, D], bf16)     # [s, e] (x16)
            wpp = pab.enter_context(tc.tile_pool(name="wpp", bufs=1))
            wp_sb = wpp.tile([P, ND, D], fp8)
            pBt = pab.enter_context(
                tc.tile_pool(name="pBt", bufs=1, side="right")
            )
            attT = pBt.tile([P, ND, T], fp8)  # [e, t] (x 1/16)

            qkscope = ExitStack()
            wqk = qkscope.enter_context(tc.tile_pool(name="wqk", bufs=1))
            psM = qkscope.enter_context(
                tc.tile_pool(name="psM", bufs=2, space="PSUM")
            )
            wq_sb = wqk.tile([P, ND, D], fp8)
            wk_sb = wqk.tile([P, ND, D], fp8)

            DR = mybir.MatmulPerfMode.DoubleRow

            def qk_proj(pr):
                for wsb, dest, bias_sb in (
                    (wq_sb, qT, qb_sb),
                    (wk_sb, kT, kb_sb),
                ):
                    for n in range(2):
                        ps = psM.tile([P, 512], f32, tag="mm")
                        for k in range(ND):
                            nc.tensor.matmul(
                                ps,
                                wsb[:, k, P * pr : P * (pr + 1)],
                                xnT[:, k, 512 * n : 512 * (n + 1)],
                                start=(k == 0),
                                stop=(k == ND - 1),
                            )
                        if bias_sb is not None:
                            nc.vector.tensor_scalar_add(
                                out=dest[:, pr, 512 * n : 512 * (n + 1)],
                                in0=ps,
                                scalar1=bias_sb[:, pr : pr + 1],
                            )
                        else:
                            nc.vector.tensor_copy(
                                out=dest[:, pr, 512 * n : 512 * (n + 1)],
                                in_=ps,
                            )

            # ============ Phase A: LN1 + transpose + QKV ============
            with ExitStack() as pa:
                stats = pa.enter_context(tc.tile_pool(name="stats", bufs=6))
                xcp = pa.enter_context(tc.tile_pool(name="xcp", bufs=2))
                psT = pa.enter_context(
                    tc.tile_pool(name="psT", bufs=4, space="PSUM")
                )
                wvp = pa.enter_context(tc.tile_pool(name="wvp", bufs=1))
                psV = pa.enter_context(
                    tc.tile_pool(name="psV", bufs=2, space="PSUM")
                )

                for i in range(NT):
                    nc.sync.dma_start(
                        out=x_sb[:, i, :], in_=x_d[P * i : P * (i + 1), :]
                    )
                    if i == 1:
                        const_dmas()
                # whole-weight loads, in consumption order, one queue
                wq_ap = wq_d[:, :].rearrange("(k p) e -> p k e", p=P)
                wk_ap = wk_d[:, :].rearrange("(k p) e -> p k e", p=P)
                wv_ap = wv_d[:, :].rearrange("(k p) e -> p k e", p=P)
                wp_ap = wp_d[:, :].rearrange("(k p) e -> p k e", p=P)
                nc.sync.dma_start(out=wq_sb, in_=wq_ap[:, :, :])
                nc.sync.dma_start(out=wk_sb, in_=wk_ap[:, :, :])
                wv_sb = wvp.tile([P, ND, D], fp8)
                nc.sync.dma_start(out=wv_sb, in_=wv_ap[:, :, :])
                nc.sync.dma_start(out=wp_sb, in_=wp_ap[:, :, :])

                for i in range(NT):
                    xc = xcp.tile([P, D], bf16, tag="xc")
                    ln_tile(stats, x_sb[:, i, :], xc, "a")
                    for j in range(ND):
                        tp = psT.tile([P, P], bf16, tag="tp")
                        nc.tensor.transpose(
                            tp, xc[:, P * j : P * (j + 1)], id_sb
                        )
                        dst = xnT[:, j, P * i : P * (i + 1)]
                        if j % 2 == 0:
                            nc.vector.tensor_copy(out=dst, in_=tp)
                        else:
                            nc.scalar.copy(out=dst, in_=tp)

                qk_proj(0)
                qk_proj(1)
                for m in range(NT):
                    for n in range(2):
                        ps = psV.tile([P, 512], f32, tag="mmv")
                        for k in range(ND):
                            nc.tensor.matmul(
                                ps,
                                xnT[:, k, P * m : P * (m + 1)],
                                wv_sb[:, k, 512 * n : 512 * (n + 1)],
                                start=(k == 0),
                                stop=(k == ND - 1),
                            )
                        if vb_sb is not None:
                            nc.vector.tensor_add(
                                out=v[:, m, 512 * n : 512 * (n + 1)],
                                in0=ps,
                                in1=vb_sb[:, 512 * n : 512 * (n + 1)],
                            )
                        else:
                            nc.vector.tensor_copy(
                                out=v[:, m, 512 * n : 512 * (n + 1)], in_=ps
                            )

            # ========= Phase B: attention (+ remaining q/k projections) ====
            with ExitStack() as pb:
                wexpp = pb.enter_context(tc.tile_pool(name="wexpp", bufs=3))
                asml = pb.enter_context(tc.tile_pool(name="asml", bufs=3))
                psS = pb.enter_context(
                    tc.tile_pool(name="psS", bufs=1, space="PSUM")
                )
                psA = pb.enter_context(
                    tc.tile_pool(name="psA", bufs=1, space="PSUM")
                )
                for pr in range(ND):  # head pairs
                    if pr >= 2:
                        qk_proj(pr)
                    attps = psA.tile([P, T], f32, tag="att")
                    sums = asml.tile([P, 2, NT], f32, tag="sums")
                    rrs = asml.tile([P, 2, NT], f32, tag="rr")
                    for i in range(NT):
                        r0 = P * i
                        rlen = T - r0
                        wexp = wexpp.tile([P, 2, T], bf16, tag="wexp")
                        spsA = psS.tile([P, rlen], f32, tag="scA")
                        spsB = psS.tile([P, rlen], f32, tag="scB")
                        sps2 = [spsA, spsB]
                        for c0 in range(0, rlen, 512):
                            cl = min(512, rlen - c0)
                            for hb in range(2):
                                base = 64 * hb
                                nc.tensor.matmul(
                                    sps2[hb][:, c0 : c0 + cl],
                                    kT[base : base + 64, pr, r0 : r0 + P],
                                    qT[
                                        base : base + 64,
                                        pr,
                                        r0 + c0 : r0 + c0 + cl,
                                    ],
                                    start=True,
                                    stop=True,
                                    tile_position=(base, 0),
                                )
                        for hb in range(2):
                            nc.vector.tensor_add(
                                out=sps2[hb][:, 0:P],
                                in0=sps2[hb][:, 0:P],
                                in1=mask_sb,
                            )
                        for hb in range(2):
                            nc.scalar.activation(
                                wexp[:, hb, r0:T],
                                sps2[hb],
                                AF.Exp,
                                scale=SCALE / (WS * WS),
                                bias=lnb_sb,
                                accum_out=sums[:, hb, i : i + 1],
                            )
                        rr = rrs[:, :, i : i + 1]
                        nc.vector.reciprocal(
                            out=rr, in_=sums[:, :, i : i + 1]
                        )
                        vp = asml.tile([P, 2, 64], bf16, tag="vp")
                        for hb in range(2):
                            base = 64 * hb
                            # vp = v_s * rr_s / 16 -> attT lands at natural
                            # scale (fp8-friendly), proj psum is then exact.
                            nc.vector.tensor_scalar(
                                out=vp[:, hb, :],
                                in0=v[:, i, P * pr + base : P * pr + base + 64],
                                scalar1=rr[:, hb, 0:1],
                                scalar2=1.0 / WS,
                                op0=ALU.mult,
                                op1=ALU.mult,
                            )
                        for lo, hi in _av_chunks(r0):
                            bank = lo // 512
                            last_i = min(NT - 1, 4 * bank + 3)
                            for hb in range(2):
                                base = 64 * hb
                                nc.tensor.matmul(
                                    attps[base : base + 64, lo:hi],
                                    vp[:, hb, :],
                                    wexp[:, hb, lo:hi],
                                    start=(i == 0),
                                    stop=(i == last_i),
                                    tile_position=(0, base),
                                )
                    nc.vector.tensor_copy(out=attT[:, pr, :], in_=attps)

            qkscope.close()

            # ============ Phase C: proj + residual + LN2 ============
            with ExitStack() as pc:
                psC = pc.enter_context(
                    tc.tile_pool(name="psC", bufs=3, space="PSUM")
                )
                psT2 = pc.enter_context(
                    tc.tile_pool(name="psT2", bufs=4, space="PSUM")
                )
                stats2 = pc.enter_context(tc.tile_pool(name="stats2", bufs=4))
                xcp2 = pc.enter_context(tc.tile_pool(name="xcp2", bufs=2))

                for m in range(NT):
                    for n in range(2):
                        pps = psC.tile([P, 512], f32, tag="mm2")
                        for k in range(ND):
                            nc.tensor.matmul(
                                pps,
                                attT[:, k, P * m : P * (m + 1)],
                                wp_sb[:, k, 512 * n : 512 * (n + 1)],
                                start=(k == 0),
                                stop=(k == ND - 1),
                            )
                        nc.vector.tensor_add(
                            out=x2[:, m, 512 * n : 512 * (n + 1)],
                            in0=x_sb[:, m, 512 * n : 512 * (n + 1)],
                            in1=pps,
                        )
                        if bp_sb is not None:
                            nc.vector.tensor_add(
                                out=x2[:, m, 512 * n : 512 * (n + 1)],
                                in0=x2[:, m, 512 * n : 512 * (n + 1)],
                                in1=bp_sb[:, 512 * n : 512 * (n + 1)],
                            )
                    xc2 = xcp2.tile([P, D], bf16, tag="xc2")
                    ln_tile(stats2, x2[:, m, :], xc2, "c")
                    for j in range(ND):
                        tp = psT2.tile([P, P], bf16, tag="tp2")
                        nc.tensor.transpose(
                            tp, xc2[:, P * j : P * (j + 1)], id_sb
                        )
                        dst = xn2T[:, j, P * m : P * (m + 1)]
                        if j % 2 == 0:
                            nc.vector.tensor_copy(out=dst, in_=tp)
                        else:
                            nc.scalar.copy(out=dst, in_=tp)

        # pab closed: attention arrays released.

        # ====== Phases D/E: FF, k-outer streaming over W1/W2 chunks ======
        w2ap = w2_d[:, :].rearrange("(k p) e -> p k e", p=P)
        with ExitStack() as pd:
            hp = pd.enter_context(tc.tile_pool(name="hp", bufs=GK + 1))
            psD = pd.enter_context(
                tc.tile_pool(name="psD", bufs=4, space="PSUM")
            )
            outp = pd.enter_context(tc.tile_pool(name="outp", bufs=4))

            for g in range(NF // GK):
                hs = []
                w2s = []
                for kk in range(GK):
                    kc = GK * g + kk
                    w1t = w1p.tile([P, ND, P], bf16, tag="w1t")
                    nc.sync.dma_start(
                        out=w1t,
                        in_=w1_d[P * kc : P * (kc + 1), :].rearrange(
                            "p (k e) -> p k e", k=ND
                        ),
                    )
                    w2t = w2p.tile([P, D], bf16, tag="w2t")
                    nc.sync.dma_start(out=w2t, in_=w2ap[:, kc, :])
                    w2s.append(w2t)
                    hc = hp.tile([P, T], bf16, tag="hc")
                    hs.append(hc)
                    for n in range(2):
                        ps = psD.tile([P, 512], f32, tag="ff1")
                        for k in range(ND):
                            nc.tensor.matmul(
                                ps,
                                w1t[:, k, :],
                                xn2T[:, k, 512 * n : 512 * (n + 1)],
                                start=(k == 0),
                                stop=(k == ND - 1),
                            )
                        if b1_sb is not None:
                            nc.scalar.activation(
                                hc[:, 512 * n : 512 * (n + 1)],
                                ps,
                                AF.Relu,
                                bias=b1_sb[:, kc : kc + 1],
                            )
                        else:
                            nc.scalar.activation(
                                hc[:, 512 * n : 512 * (n + 1)], ps, AF.Relu
                            )
                # FF2 partial for this k-group, accumulated into x2 / out
                last = g == NF // GK - 1
                for m in range(NT):
                    for n in range(2):
                        psj = psD.tile([P, 512], f32, tag="ff2")
                        for kk in range(GK):
                            nc.tensor.matmul(
                                psj,
                                hs[kk][:, P * m : P * (m + 1)],
                                w2s[kk][:, 512 * n : 512 * (n + 1)],
                                start=(kk == 0),
                                stop=(kk == GK - 1),
                            )
                        if not last:
                            nc.vector.tensor_add(
                                out=x2[:, m, 512 * n : 512 * (n + 1)],
                                in0=x2[:, m, 512 * n : 512 * (n + 1)],
                                in1=psj,
                            )
                        else:
                            ot = outp.tile([P, 512], bf16, tag="ot")
                            nc.vector.tensor_add(
                                out=ot,
                                in0=x2[:, m, 512 * n : 512 * (n + 1)],
                                in1=psj,
                            )
                            if b2_sb is not None:
                                nc.vector.tensor_add(
                                    out=ot, in0=ot,
                                    in1=b2_sb[:, 512 * n : 512 * (n + 1)],
                                )
                            nc.scalar.dma_start(
                                out=out_d[
                                    P * m : P * (m + 1),
                                    512 * n : 512 * (n + 1),
                                ],
                                in_=ot,
                            )


def _prep_inputs(
    x, gamma1, beta1, Wq, Wk, Wv, Wp, bp, gamma2, beta2, W1, b1, W2, b2
):
    g1 = np.asarray(gamma1, np.float32)
    b1n = np.asarray(beta1, np.float32)
    g2 = np.asarray(gamma2, np.float32)
    b2n = np.asarray(beta2, np.float32)
    Wq2 = np.asarray(Wq, np.float32).transpose(1, 0, 2).reshape(D, D)
    Wk2 = np.asarray(Wk, np.float32).transpose(1, 0, 2).reshape(D, D)
    Wv2 = np.asarray(Wv, np.float32).transpose(1, 0, 2).reshape(D, D)
    W1f = np.asarray(W1, np.float32)
    qb = b1n @ Wq2
    kb = b1n @ Wk2
    vb = b1n @ Wv2
    b1p = np.asarray(b1, np.float32) + b2n @ W1f

    common = {
        "wq": np.ascontiguousarray((WS * Wq2 * g1[:, None]).astype(E4M3)),
        "wk": np.ascontiguousarray((WS * Wk2 * g1[:, None]).astype(E4M3)),
        "wv": np.ascontiguousarray((WS * Wv2 * g1[:, None]).astype(E4M3)),
        "wp": np.ascontiguousarray(np.asarray(Wp, np.float32).astype(E4M3)),
        "w1": np.ascontiguousarray(
            (W1f * g2[:, None])
            .astype(BF16)
            .reshape(ND, P, NF, P)
            .transpose(2, 1, 0, 3)
            .reshape(F, D)
        ),
        "w2": np.ascontiguousarray(np.asarray(W2, np.float32).astype(BF16)),
        "mask": np.where(
            np.arange(P)[None, :] < np.arange(P)[:, None], MASKVAL, 0.0
        ).astype(np.float32),
        "ident": np.eye(P, dtype=BF16),
    }
    # zero biases are omitted from `common`; the kernel build keys off the
    # presence of these entries and skips the corresponding loads/adds.
    if np.any(b1p):
        common["b1t"] = np.ascontiguousarray(
            b1p.reshape(NF, P).T.astype(np.float32)
        )
    if np.any(qb) or np.any(kb) or np.any(vb):
        common["qbt"] = np.ascontiguousarray(
            (WS * qb).reshape(ND, P).T.astype(np.float32)
        )
        common["kbt"] = np.ascontiguousarray(
            (WS * kb).reshape(ND, P).T.astype(np.float32)
        )
        common["vbr"] = np.ascontiguousarray(
            (WS * vb).reshape(1, D).astype(np.float32)
        )
    bpf = np.asarray(bp, np.float32)
    b2f = np.asarray(b2, np.float32)
    has_bp = bool(np.any(bpf))
    has_b2 = bool(np.any(b2f))
    if has_bp:
        common["bpr"] = np.ascontiguousarray(bpf.reshape(1, D))
    if has_b2:
        common["b2r"] = np.ascontiguousarray(b2f.reshape(1, D))
    xs = np.asarray(x, np.float32).astype(BF16)
    return xs, common, has_bp, has_b2


def get_nc(reps=1, has_bp=False, has_b2=False, has_qkvb=True, has_b1=True):
    key = (reps, has_bp, has_b2, has_qkvb, has_b1)
    if key not in _cache:
        _cache[key] = _build(
            reps=reps, has_bp=has_bp, has_b2=has_b2,
            has_qkvb=has_qkvb, has_b1=has_b1,
        )
    return _cache[key]


def run(x, common, has_bp, has_b2, reps=1):
    from concourse.bass_utils import run_bass_kernel_spmd

    nc = get_nc(
        reps=reps, has_bp=has_bp, has_b2=has_b2,
        has_qkvb="qbt" in common, has_b1="b1t" in common,
    )
    in_maps = [dict(common, x=np.ascontiguousarray(x[c])) for c in range(B)]
    res = run_bass_kernel_spmd(nc, in_maps, core_ids=list(range(B)))
    return res


def kernel(x, gamma1, beta1, Wq, Wk, Wv, Wp, bp, gamma2, beta2, W1, b1, W2, b2):
    xs, common, has_bp, has_b2 = _prep_inputs(
        x, gamma1, beta1, Wq, Wk, Wv, Wp, bp, gamma2, beta2, W1, b1, W2, b2
    )
    res = run(xs, common, has_bp, has_b2, reps=1)
    out = np.stack([res.results[c]["out"] for c in range(B)], axis=0)
    return out.astype(np.float32)



# revision 21
# speedup vs baseline: 1.4310x; 1.4310x over previous
"""Trainium2 Bass kernel for nn_Block_69578470195514 (dense transformer block).

Contract: kernel(**inputs) takes the FULL unsharded inputs (B=8,T=1024,D=1024,
H=16) as numpy arrays and returns the FULL [8,1024,1024] float32 output.

Sharding: pure data-parallel over batch - core b processes batch element b.
Weights are replicated. No collectives.

The design minimizes per-iteration DRAM traffic (~22MB/core):
  x upload bf16 (2MB), out store bf16 (2MB),
  Wq/Wk/Wv/Wp stored fp8 e4m3 scaled x16 (4MB total),
  W1/W2 bf16 (16MB) streamed chunk-interleaved into a k-outer FF2 so the
  stream is consumed as it arrives (no big-resident W2, tiny compute tail).

Scale folding for the x16 fp8 weights: q_s=16q, k_s=16k (bias x16), so
scores_s=256*scores -> exp scale = D^-0.5/256. v_s=16v (bias x16); the
per-row softmax normalizer multiply also folds 1/256 so att_s=att/16;
proj with Wp_s=16*Wp then gives the exact attention projection in PSUM.

On-chip dtypes: xnT/qT/kT/attT fp8 (saves SBUF for the W1/W2 staging that
keeps the DMA queue busy during attention), v/x2/xn2T/h bf16, PSUM fp32.

v3a changes (A/B-verified ~20-30%% faster back-to-back vs the previous
version): transpose PSUM->SBUF copies batched 4-wide ([P,4,P] psum tiles,
one copy instead of four), and the QKV/V PSUM evacuation copies split
between the DVE and ACT engines instead of all-DVE.

Measured-and-rejected (slower on this hardware despite the cost model
predicting wins): fp8 DoubleRow perf mode anywhere (projections-only
variant ran 1.5x slower end-to-end), seeding the causal mask via a PE
matmul instead of DVE adds, moving w2 loads to the scalar HWDGE queue.
"""

import numpy as np
import ml_dtypes

BF16 = ml_dtypes.bfloat16
E4M3 = ml_dtypes.float8_e4m3

P = 128
B, T, D, H = 8, 1024, 1024, 16
DH = D // H
F = 4 * D
NT = T // P   # 8 token tiles
ND = D // P   # 8 feature tiles
NF = F // P   # 32 ff tiles
GK = 16       # FF2 k-group size (chunks held resident per group)
EPS = 1e-3
SCALE = float(D) ** -0.5
WS = 16.0     # fp8 weight pre-scale
MASKVAL = -1.0e6

_cache = {}


def _split_multiwait_insts(nc, mybir):
    """This walrus build allows only 1 sync-wait per instruction. Tile can
    attach several. Hoist all but the last wait of any instruction into
    preceding single-wait InstEventSemaphore carriers on the same engine."""
    for bb in nc.main_func.blocks:
        insts = list(bb.instructions)
        out = []
        changed = False
        for inst in insts:
            si = inst.sync_info
            if si is not None and si.on_wait and len(si.on_wait) > 1:
                waits = list(si.on_wait)
                for k, w in enumerate(waits[:-1]):
                    d = mybir.InstEventSemaphore(
                        name=f"{inst.name}_wsplit{k}", ins=[], outs=[]
                    )
                    d.engine = inst.engine
                    d.sync_info = mybir.SyncInfo(on_wait=[w], on_update=[])
                    out.append(d)
                inst.sync_info = mybir.SyncInfo(
                    on_wait=[waits[-1]], on_update=list(si.on_update)
                )
                changed = True
            out.append(inst)
        if changed:
            try:
                bb.instructions[:] = out
            except Exception:
                bb.instructions.clear()
                for i in out:
                    bb.add_instruction(i)


def _av_chunks(r0):
    """Column chunks for the AV matmuls of s-tile starting at r0, split on
    PSUM bank boundaries (512 fp32)."""
    chunks = []
    for b0 in range(0, T, 512):
        lo = max(r0, b0)
        hi = b0 + 512
        if lo < hi:
            chunks.append((lo, hi))
    return chunks


def _build(reps=1, has_bp=False, has_b2=False, has_qkvb=True, has_b1=True):
    from contextlib import ExitStack

    import concourse.bass as bass
    import concourse.tile as tile
    import concourse.mybir as mybir

    f32 = mybir.dt.float32
    bf16 = mybir.dt.bfloat16
    fp8 = mybir.dt.float8e4
    AF = mybir.ActivationFunctionType
    ALU = mybir.AluOpType

    nc = bass.Bass()

    x_d = nc.dram_tensor("x", [T, D], bf16, kind="ExternalInput")
    wq_d = nc.dram_tensor("wq", [D, D], fp8, kind="ExternalInput")
    wk_d = nc.dram_tensor("wk", [D, D], fp8, kind="ExternalInput")
    wv_d = nc.dram_tensor("wv", [D, D], fp8, kind="ExternalInput")
    wp_d = nc.dram_tensor("wp", [D, D], fp8, kind="ExternalInput")
    # w1 is host-chunked: w1[kc*128+p, k*128+j] = (g2*W1)[k*128+p, kc*128+j]
    # so each [P, ND*P] chunk DMA has contiguous 2KB per-partition runs.
    w1_d = nc.dram_tensor("w1", [F, D], bf16, kind="ExternalInput")
    w2_d = nc.dram_tensor("w2", [F, D], bf16, kind="ExternalInput")
    b1_d = qb_d = kb_d = vb_d = None
    if has_b1:
        b1_d = nc.dram_tensor("b1t", [P, NF], f32, kind="ExternalInput")
    if has_qkvb:
        qb_d = nc.dram_tensor("qbt", [P, ND], f32, kind="ExternalInput")
        kb_d = nc.dram_tensor("kbt", [P, ND], f32, kind="ExternalInput")
        vb_d = nc.dram_tensor("vbr", [1, D], f32, kind="ExternalInput")
    mask_d = nc.dram_tensor("mask", [P, P], bf16, kind="ExternalInput")
    id_d = nc.dram_tensor("ident", [P, P], bf16, kind="ExternalInput")
    if has_bp:
        bp_d = nc.dram_tensor("bpr", [1, D], f32, kind="ExternalInput")
    if has_b2:
        b2_d = nc.dram_tensor("b2r", [1, D], f32, kind="ExternalInput")
    out_d = nc.dram_tensor("out", [T, D], bf16, kind="ExternalOutput")

    def bcast(ap_1d):
        # [1, N] dram row -> broadcast across partitions
        return bass.AP(
            tensor=ap_1d.tensor,
            offset=ap_1d.offset,
            ap=[[0, P]] + list(ap_1d.ap)[1:],
        )

    with tile.TileContext(nc, pool_alloc_mode="queue") as tc, ExitStack() as top:
        const = top.enter_context(tc.tile_pool(name="const", bufs=1))
        mask_sb = const.tile([P, P], bf16)
        id_sb = const.tile([P, P], bf16)
        b1_sb = qb_sb = kb_sb = vb_sb = None
        if has_b1:
            b1_sb = const.tile([P, NF], f32)
        if has_qkvb:
            qb_sb = const.tile([P, ND], f32)
            kb_sb = const.tile([P, ND], f32)
            vb_sb = const.tile([P, D], f32)
        eps_sb = const.tile([P, 1], f32)
        nc.vector.memset(eps_sb, EPS)
        # exp bias -ln(WS^2): emits exp(z)/256 so wexp/sums stay in range
        # while qT/kT carry the x16 fp8 scale.
        lnb_sb = const.tile([P, 1], f32)
        nc.vector.memset(lnb_sb, -float(np.log(WS * WS)))
        bp_sb = b2_sb = None
        if has_bp:
            bp_sb = const.tile([P, D], f32)
        if has_b2:
            b2_sb = const.tile([P, D], f32)

        def const_dmas():
            nc.sync.dma_start(out=id_sb, in_=id_d[:, :])
            nc.sync.dma_start(out=mask_sb, in_=mask_d[:, :])
            if b1_sb is not None:
                nc.sync.dma_start(out=b1_sb, in_=b1_d[:, :])
            if qb_sb is not None:
                nc.sync.dma_start(out=qb_sb, in_=qb_d[:, :])
                nc.sync.dma_start(out=kb_sb, in_=kb_d[:, :])
                nc.sync.dma_start(out=vb_sb, in_=bcast(vb_d[:, :]))
            if bp_sb is not None:
                nc.sync.dma_start(out=bp_sb, in_=bcast(bp_d[:, :]))
            if b2_sb is not None:
                nc.sync.dma_start(out=b2_sb, in_=bcast(b2_d[:, :]))

        emit_args = (
            nc, tc, tile, bass, mybir, f32, bf16, fp8, AF, ALU,
            x_d, wq_d, wk_d, wv_d, wp_d, w1_d, w2_d, out_d,
            mask_sb, id_sb, b1_sb, qb_sb, kb_sb, vb_sb, eps_sb, lnb_sb,
            bp_sb, b2_sb, const_dmas,
        )
        if reps == 1:
            _emit(*emit_args)
        else:
            with tc.For_i(0, reps, 1):
                _emit(*emit_args)

    _split_multiwait_insts(nc, mybir)
    return nc


def _emit(
    nc, tc, tile, bass, mybir, f32, bf16, fp8, AF, ALU,
    x_d, wq_d, wk_d, wv_d, wp_d, w1_d, w2_d, out_d,
    mask_sb, id_sb, b1_sb, qb_sb, kb_sb, vb_sb, eps_sb, lnb_sb, bp_sb, b2_sb,
    const_dmas,
):
    from contextlib import ExitStack

    def ln_tile(stats, xin, xcout, tags):
        st = stats.tile([P, 2, 6], f32, tag=tags + "st")
        nc.vector.bn_stats(out=st[:, 0, :], in_=xin[:, 0:512])
        nc.vector.bn_stats(out=st[:, 1, :], in_=xin[:, 512:1024])
        mv = stats.tile([P, 2], f32, tag=tags + "mv")
        nc.vector.bn_aggr(out=mv, in_=st)
        sd = stats.tile([P, 1], f32, tag=tags + "sd")
        nc.scalar.activation(sd, mv[:, 1:2], AF.Sqrt, bias=eps_sb)
        rs = stats.tile([P, 1], f32, tag=tags + "rs")
        nc.vector.reciprocal(out=rs, in_=sd)
        nmu = stats.tile([P, 1], f32, tag=tags + "nmu")
        nc.vector.tensor_scalar(
            out=nmu, in0=mv[:, 0:1], scalar1=rs, scalar2=-1.0,
            op0=ALU.mult, op1=ALU.mult,
        )
        # (x - mu) * rsig on ACT: Identity(x*rs + (-mu*rs))
        nc.scalar.activation(xcout, xin, AF.Identity, bias=nmu, scale=rs)

    with ExitStack() as ctx:
        # Long-lived arrays on the RIGHT allocation stack.
        pR1 = ctx.enter_context(tc.tile_pool(name="pR1", bufs=1, side="right"))
        x2 = pR1.tile([P, NT, D], bf16)       # residual stream 2 [t, d]
        pR2 = ctx.enter_context(tc.tile_pool(name="pR2", bufs=1, side="right"))
        xn2T = pR2.tile([P, ND, T], bf16)     # ln2(x2)^T [d, t]
        pX = ctx.enter_context(tc.tile_pool(name="pX", bufs=1, side="right"))
        x_sb = pX.tile([P, NT, D], bf16)      # resident input x [t, d]

        # FF weight staging pools: sized so the DMA queue never stalls while
        # attention computes; w2 chunks of a k-group stay alive through FF2.
        w1p = ctx.enter_context(tc.tile_pool(name="w1p", bufs=8))
        w2p = ctx.enter_context(tc.tile_pool(name="w2p", bufs=GK + 2))

        # ======== phases A..C scope ========
        with ExitStack() as pab:
            pA = pab.enter_context(tc.tile_pool(name="pA", bufs=1))
            xnT = pA.tile([P, ND, T], fp8)    # ln1(x)^T  [d, t]
            qT = pA.tile([P, ND, T], fp8)     # [e, t] (x16)
            kT = pA.tile([P, ND, T], fp8)     # [e, s] (x16)
            v = pA.tile([P, NT, D], bf16)     # [s, e] (x16)
            wpp = pab.enter_context(tc.tile_pool(name="wpp", bufs=1))
            wp_sb = wpp.tile([P, ND, D], fp8)
            pBt = pab.enter_context(
                tc.tile_pool(name="pBt", bufs=1, side="right")
            )
            attT = pBt.tile([P, ND, T], fp8)  # [e, t] (x 1/16)

            qkscope = ExitStack()
            wqk = qkscope.enter_context(tc.tile_pool(name="wqk", bufs=1))
            psM = qkscope.enter_context(
                tc.tile_pool(name="psM", bufs=2, space="PSUM")
            )
            wq_sb = wqk.tile([P, ND, D], fp8)
            wk_sb = wqk.tile([P, ND, D], fp8)

            DR = mybir.MatmulPerfMode.DoubleRow

            def qk_proj(pr):
                for wsb, dest, bias_sb in (
                    (wq_sb, qT, qb_sb),
                    (wk_sb, kT, kb_sb),
                ):
                    for n in range(2):
                        ps = psM.tile([P, 512], f32, tag="mm")
                        for k in range(ND):
                            nc.tensor.matmul(
                                ps,
                                wsb[:, k, P * pr : P * (pr + 1)],
                                xnT[:, k, 512 * n : 512 * (n + 1)],
                                start=(k == 0),
                                stop=(k == ND - 1),
                            )
                        if bias_sb is not None:
                            nc.vector.tensor_scalar_add(
                                out=dest[:, pr, 512 * n : 512 * (n + 1)],
                                in0=ps,
                                scalar1=bias_sb[:, pr : pr + 1],
                            )
                        elif n == 0:
                            nc.vector.tensor_copy(
                                out=dest[:, pr, 512 * n : 512 * (n + 1)],
                                in_=ps,
                            )
                        else:
                            nc.scalar.copy(
                                out=dest[:, pr, 512 * n : 512 * (n + 1)],
                                in_=ps,
                            )

            # ============ Phase A: LN1 + transpose + QKV ============
            with ExitStack() as pa:
                stats = pa.enter_context(tc.tile_pool(name="stats", bufs=6))
                xcp = pa.enter_context(tc.tile_pool(name="xcp", bufs=2))
                psT = pa.enter_context(
                    tc.tile_pool(name="psT", bufs=4, space="PSUM")
                )
                wvp = pa.enter_context(tc.tile_pool(name="wvp", bufs=1))
                psV = pa.enter_context(
                    tc.tile_pool(name="psV", bufs=2, space="PSUM")
                )

                for i in range(NT):
                    nc.sync.dma_start(
                        out=x_sb[:, i, :], in_=x_d[P * i : P * (i + 1), :]
                    )
                    if i == 1:
                        const_dmas()
                # whole-weight loads, in consumption order, one queue
                wq_ap = wq_d[:, :].rearrange("(k p) e -> p k e", p=P)
                wk_ap = wk_d[:, :].rearrange("(k p) e -> p k e", p=P)
                wv_ap = wv_d[:, :].rearrange("(k p) e -> p k e", p=P)
                wp_ap = wp_d[:, :].rearrange("(k p) e -> p k e", p=P)
                nc.sync.dma_start(out=wq_sb, in_=wq_ap[:, :, :])
                nc.sync.dma_start(out=wk_sb, in_=wk_ap[:, :, :])
                wv_sb = wvp.tile([P, ND, D], fp8)
                nc.sync.dma_start(out=wv_sb, in_=wv_ap[:, :, :])
                nc.sync.dma_start(out=wp_sb, in_=wp_ap[:, :, :])

                for i in range(NT):
                    xc = xcp.tile([P, D], bf16, tag="xc")
                    ln_tile(stats, x_sb[:, i, :], xc, "a")
                    for a in range(2):
                        tp = psT.tile([P, 4, P], bf16, tag="tp")
                        for jj in range(4):
                            nc.tensor.transpose(
                                tp[:, jj, :],
                                xc[:, 512 * a + P * jj : 512 * a + P * (jj + 1)],
                                id_sb,
                            )
                        dst = xnT[:, 4 * a : 4 * a + 4, P * i : P * (i + 1)]
                        if a == 0:
                            nc.vector.tensor_copy(out=dst, in_=tp)
                        else:
                            nc.scalar.copy(out=dst, in_=tp)

                qk_proj(0)
                qk_proj(1)
                for m in range(NT):
                    for n in range(2):
                        ps = psV.tile([P, 512], f32, tag="mmv")
                        for k in range(ND):
                            nc.tensor.matmul(
                                ps,
                                xnT[:, k, P * m : P * (m + 1)],
                                wv_sb[:, k, 512 * n : 512 * (n + 1)],
                                start=(k == 0),
                                stop=(k == ND - 1),
                            )
                        if vb_sb is not None:
                            nc.vector.tensor_add(
                                out=v[:, m, 512 * n : 512 * (n + 1)],
                                in0=ps,
                                in1=vb_sb[:, 512 * n : 512 * (n + 1)],
                            )
                        elif n == 0:
                            nc.vector.tensor_copy(
                                out=v[:, m, 512 * n : 512 * (n + 1)], in_=ps
                            )
                        else:
                            nc.scalar.copy(
                                out=v[:, m, 512 * n : 512 * (n + 1)], in_=ps
                            )

            # ========= Phase B: attention (+ remaining q/k projections) ====
            with ExitStack() as pb:
                wexpp = pb.enter_context(tc.tile_pool(name="wexpp", bufs=3))
                asml = pb.enter_context(tc.tile_pool(name="asml", bufs=3))
                psS = pb.enter_context(
                    tc.tile_pool(name="psS", bufs=1, space="PSUM")
                )
                psA = pb.enter_context(
                    tc.tile_pool(name="psA", bufs=1, space="PSUM")
                )
                for pr in range(ND):  # head pairs
                    if pr >= 2:
                        qk_proj(pr)
                    attps = psA.tile([P, T], f32, tag="att")
                    sums = asml.tile([P, 2, NT], f32, tag="sums")
                    rrs = asml.tile([P, 2, NT], f32, tag="rr")
                    for i in range(NT):
                        r0 = P * i
                        rlen = T - r0
                        wexp = wexpp.tile([P, 2, T], bf16, tag="wexp")
                        spsA = psS.tile([P, rlen], f32, tag="scA")
                        spsB = psS.tile([P, rlen], f32, tag="scB")
                        sps2 = [spsA, spsB]
                        for c0 in range(0, rlen, 512):
                            cl = min(512, rlen - c0)
                            for hb in range(2):
                                base = 64 * hb
                                nc.tensor.matmul(
                                    sps2[hb][:, c0 : c0 + cl],
                                    kT[base : base + 64, pr, r0 : r0 + P],
                                    qT[
                                        base : base + 64,
                                        pr,
                                        r0 + c0 : r0 + c0 + cl,
                                    ],
                                    start=True,
                                    stop=True,
                                    tile_position=(base, 0),
                                )
                        for hb in range(2):
                            nc.vector.tensor_add(
                                out=sps2[hb][:, 0:P],
                                in0=sps2[hb][:, 0:P],
                                in1=mask_sb,
                            )
                        for hb in range(2):
                            nc.scalar.activation(
                                wexp[:, hb, r0:T],
                                sps2[hb],
                                AF.Exp,
                                scale=SCALE / (WS * WS),
                                bias=lnb_sb,
                                accum_out=sums[:, hb, i : i + 1],
                            )
                        rr = rrs[:, :, i : i + 1]
                        nc.vector.reciprocal(
                            out=rr, in_=sums[:, :, i : i + 1]
                        )
                        vp = asml.tile([P, 2, 64], bf16, tag="vp")
                        for hb in range(2):
                            base = 64 * hb
                            # vp = v_s * rr_s / 16 -> attT lands at natural
                            # scale (fp8-friendly), proj psum is then exact.
                            nc.vector.tensor_scalar(
                                out=vp[:, hb, :],
                                in0=v[:, i, P * pr + base : P * pr + base + 64],
                                scalar1=rr[:, hb, 0:1],
                                scalar2=1.0 / WS,
                                op0=ALU.mult,
                                op1=ALU.mult,
                            )
                        for lo, hi in _av_chunks(r0):
                            bank = lo // 512
                            last_i = min(NT - 1, 4 * bank + 3)
                            for hb in range(2):
                                base = 64 * hb
                                nc.tensor.matmul(
                                    attps[base : base + 64, lo:hi],
                                    vp[:, hb, :],
                                    wexp[:, hb, lo:hi],
                                    start=(i == 0),
                                    stop=(i == last_i),
                                    tile_position=(0, base),
                                )
                    if pr % 2 == 0:
                        nc.vector.tensor_copy(out=attT[:, pr, :], in_=attps)
                    else:
                        nc.scalar.copy(out=attT[:, pr, :], in_=attps)

            qkscope.close()

            # ============ Phase C: proj + residual + LN2 ============
            with ExitStack() as pc:
                psC = pc.enter_context(
                    tc.tile_pool(name="psC", bufs=3, space="PSUM")
                )
                psT2 = pc.enter_context(
                    tc.tile_pool(name="psT2", bufs=4, space="PSUM")
                )
                stats2 = pc.enter_context(tc.tile_pool(name="stats2", bufs=4))
                xcp2 = pc.enter_context(tc.tile_pool(name="xcp2", bufs=2))

                for m in range(NT):
                    for n in range(2):
                        pps = psC.tile([P, 512], f32, tag="mm2")
                        for k in range(ND):
                            nc.tensor.matmul(
                                pps,
                                attT[:, k, P * m : P * (m + 1)],
                                wp_sb[:, k, 512 * n : 512 * (n + 1)],
                                start=(k == 0),
                                stop=(k == ND - 1),
                            )
                        nc.vector.tensor_add(
                            out=x2[:, m, 512 * n : 512 * (n + 1)],
                            in0=x_sb[:, m, 512 * n : 512 * (n + 1)],
                            in1=pps,
                        )
                        if bp_sb is not None:
                            nc.vector.tensor_add(
                                out=x2[:, m, 512 * n : 512 * (n + 1)],
                                in0=x2[:, m, 512 * n : 512 * (n + 1)],
                                in1=bp_sb[:, 512 * n : 512 * (n + 1)],
                            )
                    xc2 = xcp2.tile([P, D], bf16, tag="xc2")
                    ln_tile(stats2, x2[:, m, :], xc2, "c")
                    for a in range(2):
                        tp = psT2.tile([P, 4, P], bf16, tag="tp2")
                        for jj in range(4):
                            nc.tensor.transpose(
                                tp[:, jj, :],
                                xc2[:, 512 * a + P * jj : 512 * a + P * (jj + 1)],
                                id_sb,
                            )
                        dst = xn2T[:, 4 * a : 4 * a + 4, P * m : P * (m + 1)]
                        if a == 0:
                            nc.vector.tensor_copy(out=dst, in_=tp)
                        else:
                            nc.scalar.copy(out=dst, in_=tp)

        # pab closed: attention arrays released.

        # ====== Phases D/E: FF, k-outer streaming over W1/W2 chunks ======
        w2ap = w2_d[:, :].rearrange("(k p) e -> p k e", p=P)
        with ExitStack() as pd:
            hp = pd.enter_context(tc.tile_pool(name="hp", bufs=GK + 1))
            psD = pd.enter_context(
                tc.tile_pool(name="psD", bufs=4, space="PSUM")
            )
            outp = pd.enter_context(tc.tile_pool(name="outp", bufs=4))

            for g in range(NF // GK):
                hs = []
                w2s = []
                for kk in range(GK):
                    kc = GK * g + kk
                    w1t = w1p.tile([P, ND, P], bf16, tag="w1t")
                    nc.sync.dma_start(
                        out=w1t,
                        in_=w1_d[P * kc : P * (kc + 1), :].rearrange(
                            "p (k e) -> p k e", k=ND
                        ),
                    )
                    w2t = w2p.tile([P, D], bf16, tag="w2t")
                    nc.sync.dma_start(out=w2t, in_=w2ap[:, kc, :])
                    w2s.append(w2t)
                    hc = hp.tile([P, T], bf16, tag="hc")
                    hs.append(hc)
                    for n in range(2):
                        ps = psD.tile([P, 512], f32, tag="ff1")
                        for k in range(ND):
                            nc.tensor.matmul(
                                ps,
                                w1t[:, k, :],
                                xn2T[:, k, 512 * n : 512 * (n + 1)],
                                start=(k == 0),
                                stop=(k == ND - 1),
                            )
                        if b1_sb is not None:
                            nc.scalar.activation(
                                hc[:, 512 * n : 512 * (n + 1)],
                                ps,
                                AF.Relu,
                                bias=b1_sb[:, kc : kc + 1],
                            )
                        else:
                            nc.scalar.activation(
                                hc[:, 512 * n : 512 * (n + 1)], ps, AF.Relu
                            )
                # FF2 partial for this k-group, accumulated into x2 / out
                last = g == NF // GK - 1
                for m in range(NT):
                    for n in range(2):
                        psj = psD.tile([P, 512], f32, tag="ff2")
                        for kk in range(GK):
                            nc.tensor.matmul(
                                psj,
                                hs[kk][:, P * m : P * (m + 1)],
                                w2s[kk][:, 512 * n : 512 * (n + 1)],
                                start=(kk == 0),
                                stop=(kk == GK - 1),
                            )
                        if not last:
                            nc.vector.tensor_add(
                                out=x2[:, m, 512 * n : 512 * (n + 1)],
                                in0=x2[:, m, 512 * n : 512 * (n + 1)],
                                in1=psj,
                            )
                        else:
                            ot = outp.tile([P, 512], bf16, tag="ot")
                            nc.vector.tensor_add(
                                out=ot,
                                in0=x2[:, m, 512 * n : 512 * (n + 1)],
                                in1=psj,
                            )
                            if b2_sb is not None:
                                nc.vector.tensor_add(
                                    out=ot, in0=ot,
                                    in1=b2_sb[:, 512 * n : 512 * (n + 1)],
                                )
                            nc.scalar.dma_start(
                                out=out_d[
                                    P * m : P * (m + 1),
                                    512 * n : 512 * (n + 1),
                                ],
                                in_=ot,
                            )


def _prep_inputs(
    x, gamma1, beta1, Wq, Wk, Wv, Wp, bp, gamma2, beta2, W1, b1, W2, b2
):
    g1 = np.asarray(gamma1, np.float32)
    b1n = np.asarray(beta1, np.float32)
    g2 = np.asarray(gamma2, np.float32)
    b2n = np.asarray(beta2, np.float32)
    Wq2 = np.asarray(Wq, np.float32).transpose(1, 0, 2).reshape(D, D)
    Wk2 = np.asarray(Wk, np.float32).transpose(1, 0, 2).reshape(D, D)
    Wv2 = np.asarray(Wv, np.float32).transpose(1, 0, 2).reshape(D, D)
    W1f = np.asarray(W1, np.float32)
    qb = b1n @ Wq2
    kb = b1n @ Wk2
    vb = b1n @ Wv2
    b1p = np.asarray(b1, np.float32) + b2n @ W1f

    common = {
        "wq": np.ascontiguousarray((WS * Wq2 * g1[:, None]).astype(E4M3)),
        "wk": np.ascontiguousarray((WS * Wk2 * g1[:, None]).astype(E4M3)),
        "wv": np.ascontiguousarray((WS * Wv2 * g1[:, None]).astype(E4M3)),
        "wp": np.ascontiguousarray(np.asarray(Wp, np.float32).astype(E4M3)),
        "w1": np.ascontiguousarray(
            (W1f * g2[:, None])
            .astype(BF16)
            .reshape(ND, P, NF, P)
            .transpose(2, 1, 0, 3)
            .reshape(F, D)
        ),
        "w2": np.ascontiguousarray(np.asarray(W2, np.float32).astype(BF16)),
        "mask": np.where(
            np.arange(P)[None, :] < np.arange(P)[:, None], MASKVAL, 0.0
        ).astype(BF16),
        "ident": np.eye(P, dtype=BF16),
    }
    # zero biases are omitted from `common`; the kernel build keys off the
    # presence of these entries and skips the corresponding loads/adds.
    if np.any(b1p):
        common["b1t"] = np.ascontiguousarray(
            b1p.reshape(NF, P).T.astype(np.float32)
        )
    if np.any(qb) or np.any(kb) or np.any(vb):
        common["qbt"] = np.ascontiguousarray(
            (WS * qb).reshape(ND, P).T.astype(np.float32)
        )
        common["kbt"] = np.ascontiguousarray(
            (WS * kb).reshape(ND, P).T.astype(np.float32)
        )
        common["vbr"] = np.ascontiguousarray(
            (WS * vb).reshape(1, D).astype(np.float32)
        )
    bpf = np.asarray(bp, np.float32)
    b2f = np.asarray(b2, np.float32)
    has_bp = bool(np.any(bpf))
    has_b2 = bool(np.any(b2f))
    if has_bp:
        common["bpr"] = np.ascontiguousarray(bpf.reshape(1, D))
    if has_b2:
        common["b2r"] = np.ascontiguousarray(b2f.reshape(1, D))
    xs = np.asarray(x, np.float32).astype(BF16)
    return xs, common, has_bp, has_b2


def get_nc(reps=1, has_bp=False, has_b2=False, has_qkvb=True, has_b1=True):
    key = (reps, has_bp, has_b2, has_qkvb, has_b1)
    if key not in _cache:
        _cache[key] = _build(
            reps=reps, has_bp=has_bp, has_b2=has_b2,
            has_qkvb=has_qkvb, has_b1=has_b1,
        )
    return _cache[key]


def run(x, common, has_bp, has_b2, reps=1):
    from concourse.bass_utils import run_bass_kernel_spmd

    nc = get_nc(
        reps=reps, has_bp=has_bp, has_b2=has_b2,
        has_qkvb="qbt" in common, has_b1="b1t" in common,
    )
    in_maps = [dict(common, x=np.ascontiguousarray(x[c])) for c in range(B)]
    res = run_bass_kernel_spmd(nc, in_maps, core_ids=list(range(B)))
    return res


def kernel(x, gamma1, beta1, Wq, Wk, Wv, Wp, bp, gamma2, beta2, W1, b1, W2, b2):
    xs, common, has_bp, has_b2 = _prep_inputs(
        x, gamma1, beta1, Wq, Wk, Wv, Wp, bp, gamma2, beta2, W1, b1, W2, b2
    )
    res = run(xs, common, has_bp, has_b2, reps=1)
    out = np.stack([res.results[c]["out"] for c in range(B)], axis=0)
    return out.astype(np.float32)

